# revision 21
# baseline (speedup 1.0000x reference)
"""Trainium2 Bass kernel for nn_MetricConv (GNN message passing with learned metric).

v2 redesign (8-core SPMD, vertex-sharded, bin-packed windows):
  out_i = sum_{e:src=i} w_e * feat[dst_e] @ W / (rowsum_i + eps) + bias
  w_e = exp(-0.5 * (u[src]+u[dst]) . p(t_e)),  u[v] = 6-vec of G[v]=M^T M

Per-core vertex table (DRAM, 512B rows): [feat bf16 (256B) | u f32 (24B) | pad].
Host packs feat-bf16 into the table input and PERMUTES rows so this core's own
vertices sit in rows 0..6272 in bin order (49 bins x 128 slots, load-balanced
by (degA, degB) so K = KA+KB is minimal). Device phase 1 computes u for every
row (bf16 MLP via DMA-transposed feature tiles) and writes u back into the
input table. Phase 2 per window: dma_gather dst rows, build one-hot via
tensor_tensor broadcast compare (fast DVE path), per-tile PE transpose for the
src-side u expansion, weighted segment-sum matmuls in bf16.
"""
import os
import numpy as np
import ml_dtypes

import concourse.bacc as bacc
import concourse.bass as bass
import concourse.tile as tile
import concourse.mybir as mybir
from concourse.bass_utils import run_bass_kernel_spmd

N, E_TOT, C, F, H = 50000, 800000, 128, 128, 32
EPS = 1e-8
NCORES = 8
NV = N // NCORES          # 6250
W = 49                    # bins/windows per core
NS = W * 128              # 6272 slots
NROWS = 50176             # table rows (49 x 1024); >= 6272 + 7*6250
HALF = 32768              # int16 gather index split point
f32 = mybir.dt.float32
bf16 = mybir.dt.bfloat16
i16 = mybir.dt.int16
AOp = mybir.AluOpType
AF = mybir.ActivationFunctionType
AxL = mybir.AxisListType

SWQ = int(os.environ.get("SWQ", "1"))  # SWDGE queues for gather DGE


def host_prep(features, vertices, edges):
    """Per-core: bin-pack srcs, permute vertex table, build gather/edge meta."""
    feats = np.asarray(features, np.float32)
    featsb = feats.astype(ml_dtypes.bfloat16)
    verts = np.asarray(vertices, np.float32)
    src = np.asarray(edges[0], np.int64).astype(np.int32)
    dst = np.asarray(edges[1], np.int64).astype(np.int32)
    t_all = verts[dst] - verts[src]
    p_all = np.empty((len(src), 6), np.float32)
    p_all[:, 0:3] = t_all * t_all
    p_all[:, 3] = t_all[:, 0] * t_all[:, 1]
    p_all[:, 4] = t_all[:, 0] * t_all[:, 2]
    p_all[:, 5] = t_all[:, 1] * t_all[:, 2]

    percore = []
    KAmax = KBmax = 1
    for c in range(NCORES):
        lo, hi = c * NV, (c + 1) * NV
        sel = np.nonzero((src >= lo) & (src < hi))[0]
        srcL = src[sel] - lo

        # --- bin-pack the 6250 srcs into 49 bins of <=128, balancing A and B
        # edge counts separately (A/B of an edge is fixed before packing: own
        # dst -> rows 0..NS < HALF -> A; other dst -> row NS + rotpos, A iff
        # rotpos < HALF - NS).
        dstE = dst[sel]
        ownD = (dstE >= lo) & (dstE < hi)
        rotpos = (dstE - hi) % N
        isAe = ownD | (rotpos < HALF - NS)
        degA = np.bincount(srcL[isAe], minlength=NV).astype(np.float64)
        degB = np.bincount(srcL[~isAe], minlength=NV).astype(np.float64)
        deg = degA + degB
        order = np.argsort(-deg, kind="stable")
        binof = np.full(NV, -1, np.int32)
        slotof = np.full(NV, -1, np.int32)
        loadsA = np.zeros(W)
        loadsB = np.zeros(W)
        cnts = np.zeros(W, np.int64)
        mA = max(degA.sum() / W, 1.0)
        mB = max(degB.sum() / W, 1.0)
        for v in order:
            cand = np.nonzero(cnts < 128)[0]
            score = np.maximum((loadsA[cand] + degA[v]) / mA,
                               (loadsB[cand] + degB[v]) / mB)
            b = cand[np.argmin(score)]
            binof[v] = b
            slotof[v] = cnts[b]
            cnts[b] += 1
            loadsA[b] += degA[v]
            loadsB[b] += degB[v]

        # vertex -> table row (this core's table)
        row_of = np.empty(N, np.int32)
        own = np.arange(lo, hi)
        row_of[own] = binof[own - lo] * 128 + slotof[own - lo]
        others = np.concatenate([np.arange(hi, N), np.arange(0, lo)])
        row_of[others] = NS + np.arange(len(others), dtype=np.int32)

        dstR = row_of[dst[sel]]
        isA = dstR < HALF
        win = binof[srcL]
        sslot = slotof[srcL]
        orderE = np.lexsort((~isA, win))
        percore.append((win[orderE], sslot[orderE], dstR[orderE], isA[orderE],
                        p_all[sel][orderE], row_of, binof, slotof))
        for w in range(W):
            m = win[orderE] == w
            na = int(np.count_nonzero(m & isA[orderE]))
            nb = int(np.count_nonzero(m)) - na
            KAmax = max(KAmax, -(-na // 128))
            KBmax = max(KBmax, -(-nb // 128))
    KA, KB = KAmax, KBmax
    K = KA + KB

    in_maps = []
    for c in range(NCORES):
        win, sslot, dstR, isA, pC, row_of, binof, slotof = percore[c]
        lo = c * NV
        # permuted feature table (packed bf16 in cols 0:64)
        inv = np.empty(NROWS, np.int64)
        inv[:] = 0
        vid = np.empty(NROWS, np.int64)
        vid[:] = -1
        # rows -> vertex
        rows = row_of  # vertex -> row
        vr = np.argsort(rows)  # vertices sorted by row
        vid[rows[vr]] = vr
        zinit = np.zeros((NROWS, 128), np.float32)
        valid = vid >= 0
        zinit[valid, 0:64] = featsb[vid[valid]].view(np.float32)

        gidx = np.zeros((W, K, 128), np.int16)
        srcrel = np.full((W, K, 128), 200.0, np.float32)
        pbuf = np.zeros((W, 128, K, 6), np.float32)
        for w in range(W):
            m = win == w
            for grp in range(2):
                g = m & (isA if grp == 0 else ~isA)
                idxs = np.nonzero(g)[0]
                n = len(idxs)
                if n == 0:
                    continue
                t0 = 0 if grp == 0 else KA
                off = 0 if grp == 0 else HALF
                ntile = -(-n // 128)
                pad = ntile * 128 - n
                di = np.concatenate([dstR[idxs] - off, np.zeros(pad, np.int32)])
                sr = np.concatenate([sslot[idxs],
                                     np.full(pad, 200, np.int32)]).astype(np.float32)
                pv = np.concatenate([pC[idxs], np.zeros((pad, 6), np.float32)], 0)
                gidx[w, t0:t0 + ntile] = di.reshape(ntile, 128).astype(np.int16)
                srcrel[w, t0:t0 + ntile] = sr.reshape(ntile, 128)
                pbuf[w, :, t0:t0 + ntile, :] = pv.reshape(ntile, 128, 6).transpose(1, 0, 2)

        def wrap(g2, ntile):
            n = ntile * 128
            gw = g2.reshape(W, n // 16, 16).transpose(0, 2, 1)
            return np.ascontiguousarray(np.tile(gw, (1, 8, 1)))
        gm = np.concatenate([wrap(gidx[:, :KA].reshape(W, KA * 128), KA),
                             wrap(gidx[:, KA:].reshape(W, KB * 128), KB)], axis=2)
        # out row (w*128+slot) -> vertex id (or -1)
        outvert = np.full(NS, -1, np.int64)
        ownv = np.arange(lo, lo + NV)
        outvert[rows[ownv]] = ownv
        in_maps.append({
            "ztab": zinit,
            "emp": np.ascontiguousarray(
                pbuf.reshape(W, 128, K * 6)),                    # [W,128,K*6] f32
            "ems": np.ascontiguousarray(
                srcrel.transpose(0, 2, 1)).astype(ml_dtypes.bfloat16),  # [W,128,K]
            "gidx": np.ascontiguousarray(gm),                    # [W,128,K*8] i16
        })
        in_maps[-1]["_outvert"] = outvert
    return in_maps, KA, KB


def const_inputs(W1, b1, W2, b2, weights, bias, K):
    ii = np.tile(np.arange(128, dtype=np.float32), K)
    return {
        "w1b": np.asarray(W1, np.float32).astype(ml_dtypes.bfloat16),   # [128,32]
        "b1c": np.asarray(b1, np.float32).reshape(H, 1),                # [32,1]
        "w2b": np.asarray(W2, np.float32).astype(ml_dtypes.bfloat16),   # [32,9]
        "b2bc": np.tile(np.asarray(b2, np.float32), (128, 8)),          # [128,72]
        "wt": np.asarray(weights, np.float32),                          # [128,128]
        "biasbc": np.tile(np.asarray(bias, np.float32), (128, 1)),      # [128,128]
        "identb": np.eye(128, dtype=np.float32).astype(ml_dtypes.bfloat16),
        "ident1": np.eye(1, dtype=np.float32),
        "iotak": np.tile(ii, (128, 1)).astype(ml_dtypes.bfloat16),      # [128,K*128]
        "onesc": np.ones((128, 1), np.float32).astype(ml_dtypes.bfloat16),
    }


def const_shapes(K):
    return {"w1b": ([C, H], bf16), "b1c": ([H, 1], f32), "w2b": ([H, 9], bf16),
            "b2bc": ([128, 72], f32), "wt": ([C, F], f32),
            "biasbc": ([128, F], f32), "identb": ([128, 128], bf16),
            "ident1": ([1, 1], f32),
            "iotak": ([128, K * 128], bf16), "onesc": ([128, 1], bf16)}


def build_nc(KA, KB):
    K = KA + KB
    nc = bacc.Bacc("TRN2", target_bir_lowering=False, debug=False,
                   num_devices=NCORES, num_swdge_queues=SWQ)

    d_z = nc.dram_tensor("ztab", [NROWS, C], f32, kind="ExternalInput")
    d_emp = nc.dram_tensor("emp", [W, 128, K * 6], f32, kind="ExternalInput")
    d_ems = nc.dram_tensor("ems", [W, 128, K], bf16, kind="ExternalInput")
    d_gi = nc.dram_tensor("gidx", [W, 128, K * 8], i16, kind="ExternalInput")
    CS = const_shapes(K)
    dc = {k: nc.dram_tensor(k, sh, dt, kind="ExternalInput")
          for k, (sh, dt) in CS.items()}
    d_out = nc.dram_tensor("out", [NS, F], f32, kind="ExternalOutput")
    zb = d_z.ap().bitcast(bf16)  # [NROWS, 256] bf16 view

    with tile.TileContext(nc) as tc:
        with tc.tile_pool(name="const", bufs=1) as cp:
            cs = {}
            for k, (sh, dt) in CS.items():
                cs[k] = cp.tile(sh, dt, tag=k, name=k)
                nc.sync.dma_start(cs[k][:, :], dc[k].ap()[:, :])

            # ---------------- Phase 1: compute u for all rows ----------------
            with tc.tile_pool(name="p1", bufs=3) as p1, \
                 tc.tile_pool(name="p1f", bufs=1) as p1f, \
                 tc.tile_pool(name="p1b", bufs=2) as p1b, \
                 tc.tile_pool(name="ps1", bufs=2, space="PSUM") as ps1, \
                 tc.tile_pool(name="psm", bufs=2, space="PSUM") as psm:
                # all transposed feature reads issue before any u-write so the
                # chunks pipeline instead of serializing on the z table
                ftall = p1f.tile([128, NROWS], bf16, tag="ftall", name="ftall")
                for chunk in range(NROWS // 1024):
                    v0 = chunk * 1024
                    nc.sync.dma_start_transpose(ftall[:, v0:v0 + 1024],
                                                zb[v0:v0 + 1024, 0:128])
                for chunk in range(NROWS // 1024):
                    v0 = chunk * 1024
                    ftT = ftall[:, v0:v0 + 1024]
                    mb = psm.tile([128, 72], f32, tag="mb", name="mb")
                    for hf in range(2):
                        hT_ps = ps1.tile([32, 512], f32, tag="hT", name="hT")
                        nc.tensor.matmul(hT_ps[:, :], cs["w1b"][:, :],
                                         ftT[:, hf * 512:(hf + 1) * 512],
                                         start=True, stop=True)
                        hTs = p1.tile([32, 512], bf16, tag="hTs", name="hTs")
                        nc.scalar.activation(hTs[:, :], hT_ps[:, :], AF.Relu,
                                             bias=cs["b1c"][:, :])
                        for g in range(4):
                            gg = hf * 4 + g
                            nc.tensor.matmul(mb[:, gg * 9:gg * 9 + 9],
                                             hTs[:, g * 128:(g + 1) * 128],
                                             cs["w2b"][:, :], start=True, stop=True)
                    # u from M batch: G = M^T M -> 6-vector (crosses doubled)
                    m_s = p1b.tile([128, 72], f32, tag="m", name="m")
                    nc.vector.tensor_add(m_s[:, :], mb[:, :], cs["b2bc"][:, :])
                    sq = p1b.tile([128, 72], f32, tag="sq", name="sq")
                    nc.vector.tensor_mul(sq[:, :], m_s[:, :], m_s[:, :])
                    u_t = p1b.tile([128, 48], f32, tag="u", name="u")
                    u3 = u_t[:, :].rearrange("p (g c) -> p g c", c=6)
                    s3 = sq[:, :].rearrange("p (g c) -> p g c", c=9)
                    nc.vector.tensor_add(u3[:, :, 0:3], s3[:, :, 0:3], s3[:, :, 3:6])
                    nc.vector.tensor_add(u3[:, :, 0:3], u3[:, :, 0:3], s3[:, :, 6:9])
                    m4 = m_s[:, :].rearrange("p (g k i) -> p g k i", k=3, i=3)
                    u4 = u_t[:, :].rearrange("p (g c i) -> p g c i", c=6, i=1)
                    ct = p1b.tile([128, 24], f32, tag="ct", name="ct")
                    ct4 = ct[:, :].rearrange("p (g k i) -> p g k i", k=3, i=1)
                    for ci, (i, j) in enumerate([(0, 1), (0, 2), (1, 2)]):
                        nc.vector.tensor_mul(ct4[:, :, :, :], m4[:, :, :, i:i + 1],
                                             m4[:, :, :, j:j + 1])
                        nc.vector.tensor_add(u4[:, :, 3 + ci:4 + ci, :],
                                             ct4[:, :, 0:1, :], ct4[:, :, 1:2, :])
                        nc.vector.tensor_add(u4[:, :, 3 + ci:4 + ci, :],
                                             u4[:, :, 3 + ci:4 + ci, :],
                                             ct4[:, :, 2:3, :])
                    nc.vector.tensor_scalar_mul(u3[:, :, 3:6], u3[:, :, 3:6], 2.0)
                    nc.sync.dma_start(
                        d_z.ap()[v0:v0 + 1024, 64:70].rearrange(
                            "(g p) c -> p g c", p=128),
                        u3[:, :, :])

            # ---------------- Phase 2: edge windows ----------------
            nwin = 0 if os.environ.get("SKIP_P2") else W
            with tc.tile_pool(name="p2", bufs=3) as p2, \
                 tc.tile_pool(name="p2g", bufs=2) as p2g, \
                 tc.tile_pool(name="p2w", bufs=2) as p2w, \
                 tc.tile_pool(name="pstr", bufs=2, space="PSUM") as pstr, \
                 tc.tile_pool(name="psus", bufs=2, space="PSUM") as psus, \
                 tc.tile_pool(name="psag", bufs=2, space="PSUM") as psag, \
                 tc.tile_pool(name="psrs", bufs=1, space="PSUM") as psrs, \
                 tc.tile_pool(name="pse", bufs=1, space="PSUM") as pse:
                for w in range(nwin):
                    emp = p2w.tile([128, K * 6], f32, tag="emp", name="emp")
                    nc.sync.dma_start(emp[:, :], d_emp.ap()[w, :, :])
                    p3 = emp[:, :].rearrange("p (k c) -> p k c", c=6)
                    ems = p2w.tile([128, K], bf16, tag="ems", name="ems")
                    nc.sync.dma_start(ems[:, :], d_ems.ap()[w, :, :])
                    gi = p2w.tile([128, K * 8], i16, tag="gi", name="gi")
                    nc.sync.dma_start(gi[:, :], d_gi.ap()[w, :, :])
                    gia = gi[:, 0:KA * 8]
                    gib = gi[:, KA * 8:K * 8]
                    vwin = p2w.tile([128, 6], f32, tag="vwin", name="vwin")
                    nc.sync.dma_start(vwin[:, :], d_z.ap()[w * 128:w * 128 + 128, 64:70])
                    vwinb = p2w.tile([128, 6], bf16, tag="vwinb", name="vwinb")
                    nc.vector.tensor_copy(vwinb[:, :], vwin[:, :])

                    gbuf = p2g.tile([128, K, 128], f32, tag="gbuf", name="gbuf")
                    CH = 8
                    for c0 in range(0, KA, CH):
                        n = min(CH, KA - c0)
                        nc.gpsimd.dma_gather(
                            gbuf[:, c0:c0 + n, :], d_z.ap()[:, :],
                            gia[:, c0 * 8:(c0 + n) * 8], n * 128, n * 128, 128,
                            queue_num=(c0 // CH) % SWQ)
                    for c0 in range(0, KB, CH):
                        n = min(CH, KB - c0)
                        nc.gpsimd.dma_gather(
                            gbuf[:, KA + c0:KA + c0 + n, :], d_z.ap()[HALF:, :],
                            gib[:, c0 * 8:(c0 + n) * 8], n * 128, n * 128, 128,
                            queue_num=(KA // CH + c0 // CH + 1) % SWQ)

                    # one-hot [slot, K, src] in bf16 via broadcast compare
                    oh = p2w.tile([128, K, 128], bf16, tag="oh", name="oh")
                    srCb = ems[:, :].rearrange("p (k o) -> p k o", o=1)
                    nc.vector.tensor_tensor(
                        oh[:, :, :], cs["iotak"][:, :].rearrange(
                            "p (k s) -> p k s", s=128),
                        srCb.broadcast_to([128, K, 128]), AOp.is_equal)

                    # src-side u expansion: transpose one-hots, us = s01T @ vwin
                    us_ps = psus.tile([128, K * 6], f32, tag="us", name="us")
                    nb4 = -(-K // 4)
                    for b4 in range(nb4):
                        t0 = b4 * 4
                        nt = min(4, K - t0)
                        sT_ps = pstr.tile([128, 512], bf16, tag="sT", name="sT")
                        for t in range(nt):
                            nc.tensor.transpose(sT_ps[:, t * 128:(t + 1) * 128],
                                                oh[:, t0 + t, :], cs["identb"][:, :])
                        sT_sb = p2.tile([128, 512], bf16, tag="sTs", name="sTs")
                        nc.scalar.copy(sT_sb[:, 0:nt * 128], sT_ps[:, 0:nt * 128])
                        for t in range(nt):
                            nc.tensor.matmul(
                                us_ps[:, (t0 + t) * 6:(t0 + t) * 6 + 6],
                                sT_sb[:, t * 128:(t + 1) * 128], vwinb[:, :],
                                start=True, stop=True)

                    # q = sum_c (us + u_dst) * p ; w = exp(-0.5 q)
                    usum = p2w.tile([128, K * 6], f32, tag="usum", name="usum")
                    us3 = usum[:, :].rearrange("p (k c) -> p k c", c=6)
                    nc.vector.tensor_add(
                        us3[:, :, :],
                        us_ps[:, :].rearrange("p (k c) -> p k c", c=6),
                        gbuf[:, :, 64:70])
                    pu = p2w.tile([128, K * 6], f32, tag="pu", name="pu")
                    pu3 = pu[:, :].rearrange("p (k c) -> p k c", c=6)
                    nc.vector.tensor_mul(pu3[:, :, :], us3[:, :, :], p3[:, :, :])
                    qcol = p2w.tile([128, K], f32, tag="qcol", name="qcol")
                    nc.vector.tensor_reduce(
                        qcol[:, :].rearrange("p (k o) -> p k o", o=1),
                        pu3[:, :, :], AxL.X, AOp.add)
                    wcolb = p2w.tile([128, K], bf16, tag="wcolb", name="wcolb")
                    nc.scalar.activation(wcolb[:, :], qcol[:, :], AF.Exp, scale=-0.5)

                    # sw = one-hot * w  (bf16)
                    sw = p2w.tile([128, K, 128], bf16, tag="sw", name="sw")
                    wcb = wcolb[:, :].rearrange("p (k o) -> p k o", o=1)
                    nc.vector.tensor_tensor(sw[:, :, :], oh[:, :, :],
                                            wcb.broadcast_to([128, K, 128]), AOp.mult)

                    # weighted segment-sum in [feat, src] layout + rowsum
                    aggT = psag.tile([128, 128], f32, tag="aggT", name="aggT")
                    rs_ps = psrs.tile([1, 128], f32, tag="rs", name="rs")
                    for t in range(K):
                        gf = gbuf[:, t, 0:64].bitcast(bf16)
                        nc.tensor.matmul(aggT[:, :], gf, sw[:, t, :],
                                         start=(t == 0), stop=(t == K - 1))
                        nc.tensor.matmul(rs_ps[:, :], cs["onesc"][:, :], sw[:, t, :],
                                         start=(t == 0), stop=(t == K - 1))

                    # epilogue: out = rcp * (aggT.T @ Wt) + bias
                    aggTs = p2.tile([128, 128], f32, tag="aggTs", name="aggTs")
                    nc.scalar.copy(aggTs[:, :], aggT[:, :])
                    rs_sb = p2.tile([1, 128], f32, tag="rs_sb", name="rs_sb")
                    nc.scalar.copy(rs_sb[:, :], rs_ps[:, :])
                    rsT_ps = pse.tile([128, 128], f32, tag="pse_t", name="rsT_ps")
                    nc.tensor.transpose(rsT_ps[:, 0:1], rs_sb[:, :],
                                        cs["ident1"][:, :])
                    rse = p2.tile([128, 1], f32, tag="rse", name="rse")
                    nc.vector.tensor_scalar_add(rse[:, :], rsT_ps[:, 0:1], EPS)
                    rcp = p2.tile([128, 1], f32, tag="rcp", name="rcp")
                    nc.vector.reciprocal(rcp[:, :], rse[:, :])
                    out_ps = pse.tile([128, 128], f32, tag="pse_t", name="out_ps")
                    nc.tensor.matmul(out_ps[:, :], aggTs[:, :], cs["wt"][:, :],
                                     start=True, stop=True)
                    out_s = p2.tile([128, 128], f32, tag="outs", name="outs")
                    nc.vector.tensor_mul(out_s[:, :], out_ps[:, :],
                                         rcp[:, :].broadcast_to([128, 128]))
                    nc.vector.tensor_add(out_s[:, :], out_s[:, :], cs["biasbc"][:, :])
                    nc.sync.dma_start(d_out.ap()[w * 128:(w + 1) * 128, :], out_s[:, :])

    nc.compile()
    return nc


_cache = {}


def kernel(features, vertices, edges, faces, W1, b1, W2, b2, weights, bias,
           _trace=False):
    in_maps, KA, KB = host_prep(features, vertices, edges)
    consts = const_inputs(W1, b1, W2, b2, weights, bias, KA + KB)
    outverts = []
    for m in in_maps:
        outverts.append(m.pop("_outvert"))
        m.update(consts)
    key = (KA, KB)
    if key not in _cache:
        _cache[key] = build_nc(KA, KB)
    nc = _cache[key]
    res = run_bass_kernel_spmd(nc, in_maps, core_ids=list(range(NCORES)),
                               trace=_trace)
    out = np.empty((N, F), np.float32)
    for c in range(NCORES):
        o = res.results[c]["out"]
        ov = outverts[c]
        valid = ov >= 0
        out[ov[valid]] = o[valid]
    kernel.last_result = res
    return out


# revision 23
# speedup vs baseline: 1.0644x; 1.0644x over previous
"""Trainium2 Bass kernel for nn_MetricConv (GNN message passing with learned metric).

v2 redesign (8-core SPMD, vertex-sharded, bin-packed windows):
  out_i = sum_{e:src=i} w_e * feat[dst_e] @ W / (rowsum_i + eps) + bias
  w_e = exp(-0.5 * (u[src]+u[dst]) . p(t_e)),  u[v] = 6-vec of G[v]=M^T M

Per-core vertex table (DRAM, 512B rows): [feat bf16 (256B) | u f32 (24B) | pad].
Host packs feat-bf16 into the table input and PERMUTES rows so this core's own
vertices sit in rows 0..6272 in bin order (49 bins x 128 slots, load-balanced
by (degA, degB) so K = KA+KB is minimal). Device phase 1 computes u for every
row (bf16 MLP via DMA-transposed feature tiles) and writes u back into the
input table. Phase 2 per window: dma_gather dst rows, build one-hot via
tensor_tensor broadcast compare (fast DVE path), per-tile PE transpose for the
src-side u expansion, weighted segment-sum matmuls in bf16.
"""
import os
import numpy as np
import ml_dtypes

import concourse.bacc as bacc
import concourse.bass as bass
import concourse.tile as tile
import concourse.mybir as mybir
from concourse.bass_utils import run_bass_kernel_spmd

N, E_TOT, C, F, H = 50000, 800000, 128, 128, 32
EPS = 1e-8
NCORES = 8
NV = N // NCORES          # 6250
W = 49                    # bins/windows per core
NS = W * 128              # 6272 slots
NROWS = 50176             # table rows (49 x 1024); >= 6272 + 7*6250
HALF = 32768              # int16 gather index split point
f32 = mybir.dt.float32
bf16 = mybir.dt.bfloat16
i16 = mybir.dt.int16
AOp = mybir.AluOpType
AF = mybir.ActivationFunctionType
AxL = mybir.AxisListType

SWQ = int(os.environ.get("SWQ", "1"))  # SWDGE queues for gather DGE


def host_prep(features, vertices, edges):
    """Per-core: bin-pack srcs, permute vertex table, build gather/edge meta."""
    feats = np.asarray(features, np.float32)
    featsb = feats.astype(ml_dtypes.bfloat16)
    verts = np.asarray(vertices, np.float32)
    src = np.asarray(edges[0], np.int64).astype(np.int32)
    dst = np.asarray(edges[1], np.int64).astype(np.int32)
    t_all = verts[dst] - verts[src]
    p_all = np.empty((len(src), 6), np.float32)
    p_all[:, 0:3] = t_all * t_all
    p_all[:, 3] = t_all[:, 0] * t_all[:, 1]
    p_all[:, 4] = t_all[:, 0] * t_all[:, 2]
    p_all[:, 5] = t_all[:, 1] * t_all[:, 2]

    percore = []
    KAmax = KBmax = 1
    for c in range(NCORES):
        lo, hi = c * NV, (c + 1) * NV
        sel = np.nonzero((src >= lo) & (src < hi))[0]
        srcL = src[sel] - lo

        # --- bin-pack the 6250 srcs into 49 bins of <=128, balancing A and B
        # edge counts separately (A/B of an edge is fixed before packing: own
        # dst -> rows 0..NS < HALF -> A; other dst -> row NS + rotpos, A iff
        # rotpos < HALF - NS).
        dstE = dst[sel]
        ownD = (dstE >= lo) & (dstE < hi)
        rotpos = (dstE - hi) % N
        isAe = ownD | (rotpos < HALF - NS)
        degA = np.bincount(srcL[isAe], minlength=NV).astype(np.float64)
        degB = np.bincount(srcL[~isAe], minlength=NV).astype(np.float64)
        deg = degA + degB
        order = np.argsort(-deg, kind="stable")
        binof = np.full(NV, -1, np.int32)
        slotof = np.full(NV, -1, np.int32)
        loadsA = np.zeros(W)
        loadsB = np.zeros(W)
        cnts = np.zeros(W, np.int64)
        mA = max(degA.sum() / W, 1.0)
        mB = max(degB.sum() / W, 1.0)
        for v in order:
            cand = np.nonzero(cnts < 128)[0]
            score = np.maximum((loadsA[cand] + degA[v]) / mA,
                               (loadsB[cand] + degB[v]) / mB)
            b = cand[np.argmin(score)]
            binof[v] = b
            slotof[v] = cnts[b]
            cnts[b] += 1
            loadsA[b] += degA[v]
            loadsB[b] += degB[v]

        # vertex -> table row (this core's table)
        row_of = np.empty(N, np.int32)
        own = np.arange(lo, hi)
        row_of[own] = binof[own - lo] * 128 + slotof[own - lo]
        others = np.concatenate([np.arange(hi, N), np.arange(0, lo)])
        row_of[others] = NS + np.arange(len(others), dtype=np.int32)

        dstR = row_of[dst[sel]]
        isA = dstR < HALF
        win = binof[srcL]
        sslot = slotof[srcL]
        orderE = np.lexsort((~isA, win))
        percore.append((win[orderE], sslot[orderE], dstR[orderE], isA[orderE],
                        p_all[sel][orderE], row_of, binof, slotof))
        for w in range(W):
            m = win[orderE] == w
            na = int(np.count_nonzero(m & isA[orderE]))
            nb = int(np.count_nonzero(m)) - na
            KAmax = max(KAmax, -(-na // 128))
            KBmax = max(KBmax, -(-nb // 128))
    KA, KB = KAmax, KBmax
    K = KA + KB

    in_maps = []
    for c in range(NCORES):
        win, sslot, dstR, isA, pC, row_of, binof, slotof = percore[c]
        lo = c * NV
        # permuted feature table (packed bf16 in cols 0:64)
        vid = np.empty(NROWS, np.int64)
        vid[:] = -1
        # rows -> vertex
        rows = row_of  # vertex -> row
        vr = np.argsort(rows)  # vertices sorted by row
        vid[rows[vr]] = vr
        zinit = np.zeros((NROWS, 128), np.float32)
        valid = vid >= 0
        zinit[valid, 0:64] = featsb[vid[valid]].view(np.float32)

        gidx = np.zeros((W, K, 128), np.int16)
        srcrel = np.full((W, K, 128), 200.0, np.float32)
        pbuf = np.zeros((W, 128, K, 6), np.float32)
        for w in range(W):
            m = win == w
            for grp in range(2):
                g = m & (isA if grp == 0 else ~isA)
                idxs = np.nonzero(g)[0]
                n = len(idxs)
                if n == 0:
                    continue
                t0 = 0 if grp == 0 else KA
                off = 0 if grp == 0 else HALF
                ntile = -(-n // 128)
                pad = ntile * 128 - n
                di = np.concatenate([dstR[idxs] - off, np.zeros(pad, np.int32)])
                sr = np.concatenate([sslot[idxs],
                                     np.full(pad, 200, np.int32)]).astype(np.float32)
                pv = np.concatenate([pC[idxs], np.zeros((pad, 6), np.float32)], 0)
                gidx[w, t0:t0 + ntile] = di.reshape(ntile, 128).astype(np.int16)
                srcrel[w, t0:t0 + ntile] = sr.reshape(ntile, 128)
                pbuf[w, :, t0:t0 + ntile, :] = pv.reshape(ntile, 128, 6).transpose(1, 0, 2)

        def wrap(g2, ntile):
            n = ntile * 128
            gw = g2.reshape(W, n // 16, 16).transpose(0, 2, 1)
            return np.ascontiguousarray(np.tile(gw, (1, 8, 1)))
        gm = np.concatenate([wrap(gidx[:, :KA].reshape(W, KA * 128), KA),
                             wrap(gidx[:, KA:].reshape(W, KB * 128), KB)], axis=2)
        # out row (w*128+slot) -> vertex id (or -1)
        outvert = np.full(NS, -1, np.int64)
        ownv = np.arange(lo, lo + NV)
        outvert[rows[ownv]] = ownv
        in_maps.append({
            "ztab": zinit,
            "emp": np.ascontiguousarray(
                pbuf.reshape(W, 128, K * 6)),                    # [W,128,K*6] f32
            "ems": np.ascontiguousarray(
                srcrel.transpose(0, 2, 1)).astype(ml_dtypes.bfloat16),  # [W,128,K]
            "gidx": np.ascontiguousarray(gm),                    # [W,128,K*8] i16
        })
        in_maps[-1]["_outvert"] = outvert
    return in_maps, KA, KB


def const_inputs(W1, b1, W2, b2, weights, bias, K):
    ii = np.tile(np.arange(128, dtype=np.float32), K)
    return {
        "w1b": np.asarray(W1, np.float32).astype(ml_dtypes.bfloat16),   # [128,32]
        "b1c": np.asarray(b1, np.float32).reshape(H, 1),                # [32,1]
        "w2b": np.asarray(W2, np.float32).astype(ml_dtypes.bfloat16),   # [32,9]
        "b2bc": np.tile(np.asarray(b2, np.float32), (128, 8)),          # [128,72]
        "wt": np.asarray(weights, np.float32),                          # [128,128]
        "biasbc": np.tile(np.asarray(bias, np.float32), (128, 1)),      # [128,128]
        "identb": np.eye(128, dtype=np.float32).astype(ml_dtypes.bfloat16),
        "ident1": np.eye(1, dtype=np.float32),
        "iotak": np.tile(ii, (128, 1)).astype(ml_dtypes.bfloat16),      # [128,K*128]
        "onesc": np.ones((128, 1), np.float32).astype(ml_dtypes.bfloat16),
    }


def const_shapes(K):
    return {"w1b": ([C, H], bf16), "b1c": ([H, 1], f32), "w2b": ([H, 9], bf16),
            "b2bc": ([128, 72], f32), "wt": ([C, F], f32),
            "biasbc": ([128, F], f32), "identb": ([128, 128], bf16),
            "ident1": ([1, 1], f32),
            "iotak": ([128, K * 128], bf16), "onesc": ([128, 1], bf16)}


def build_nc(KA, KB):
    K = KA + KB
    nc = bacc.Bacc("TRN2", target_bir_lowering=False, debug=False,
                   num_devices=NCORES, num_swdge_queues=SWQ)

    d_z = nc.dram_tensor("ztab", [NROWS, C], f32, kind="ExternalInput")
    d_emp = nc.dram_tensor("emp", [W, 128, K * 6], f32, kind="ExternalInput")
    d_ems = nc.dram_tensor("ems", [W, 128, K], bf16, kind="ExternalInput")
    d_gi = nc.dram_tensor("gidx", [W, 128, K * 8], i16, kind="ExternalInput")
    CS = const_shapes(K)
    dc = {k: nc.dram_tensor(k, sh, dt, kind="ExternalInput")
          for k, (sh, dt) in CS.items()}
    d_out = nc.dram_tensor("out", [NS, F], f32, kind="ExternalOutput")
    zb = d_z.ap().bitcast(bf16)  # [NROWS, 256] bf16 view

    with tile.TileContext(nc) as tc:
        with tc.tile_pool(name="const", bufs=1) as cp:
            cs = {}
            for k, (sh, dt) in CS.items():
                cs[k] = cp.tile(sh, dt, tag=k, name=k)
                nc.sync.dma_start(cs[k][:, :], dc[k].ap()[:, :])

            # ---------------- Phase 1: compute u for all rows ----------------
            with tc.tile_pool(name="p1", bufs=3) as p1, \
                 tc.tile_pool(name="p1b", bufs=2) as p1b, \
                 tc.tile_pool(name="ps1", bufs=2, space="PSUM") as ps1, \
                 tc.tile_pool(name="psm", bufs=2, space="PSUM") as psm:
                for chunk in range(NROWS // 1024):
                    v0 = chunk * 1024
                    ftTt = p1.tile([128, 1024], bf16, tag="ftT", name="ftT")
                    nc.sync.dma_start_transpose(ftTt[:, :], zb[v0:v0 + 1024, 0:128])
                    ftT = ftTt[:, :]
                    mb = psm.tile([128, 72], f32, tag="mb", name="mb")
                    for hf in range(2):
                        hT_ps = ps1.tile([32, 512], f32, tag="hT", name="hT")
                        nc.tensor.matmul(hT_ps[:, :], cs["w1b"][:, :],
                                         ftT[:, hf * 512:(hf + 1) * 512],
                                         start=True, stop=True)
                        hTs = p1.tile([32, 512], bf16, tag="hTs", name="hTs")
                        nc.scalar.activation(hTs[:, :], hT_ps[:, :], AF.Relu,
                                             bias=cs["b1c"][:, :])
                        for g in range(4):
                            gg = hf * 4 + g
                            nc.tensor.matmul(mb[:, gg * 9:gg * 9 + 9],
                                             hTs[:, g * 128:(g + 1) * 128],
                                             cs["w2b"][:, :], start=True, stop=True)
                    # u from M batch: G = M^T M -> 6-vector (crosses doubled)
                    m_s = p1b.tile([128, 72], f32, tag="m", name="m")
                    nc.vector.tensor_add(m_s[:, :], mb[:, :], cs["b2bc"][:, :])
                    sq = p1b.tile([128, 72], f32, tag="sq", name="sq")
                    nc.vector.tensor_mul(sq[:, :], m_s[:, :], m_s[:, :])
                    u_t = p1b.tile([128, 48], f32, tag="u", name="u")
                    u3 = u_t[:, :].rearrange("p (g c) -> p g c", c=6)
                    s3 = sq[:, :].rearrange("p (g c) -> p g c", c=9)
                    nc.vector.tensor_add(u3[:, :, 0:3], s3[:, :, 0:3], s3[:, :, 3:6])
                    nc.vector.tensor_add(u3[:, :, 0:3], u3[:, :, 0:3], s3[:, :, 6:9])
                    m4 = m_s[:, :].rearrange("p (g k i) -> p g k i", k=3, i=3)
                    u4 = u_t[:, :].rearrange("p (g c i) -> p g c i", c=6, i=1)
                    ct = p1b.tile([128, 24], f32, tag="ct", name="ct")
                    ct4 = ct[:, :].rearrange("p (g k i) -> p g k i", k=3, i=1)
                    for ci, (i, j) in enumerate([(0, 1), (0, 2), (1, 2)]):
                        nc.vector.tensor_mul(ct4[:, :, :, :], m4[:, :, :, i:i + 1],
                                             m4[:, :, :, j:j + 1])
                        nc.vector.tensor_add(u4[:, :, 3 + ci:4 + ci, :],
                                             ct4[:, :, 0:1, :], ct4[:, :, 1:2, :])
                        nc.vector.tensor_add(u4[:, :, 3 + ci:4 + ci, :],
                                             u4[:, :, 3 + ci:4 + ci, :],
                                             ct4[:, :, 2:3, :])
                    nc.vector.tensor_scalar_mul(u3[:, :, 3:6], u3[:, :, 3:6], 2.0)
                    nc.sync.dma_start(
                        d_z.ap()[v0:v0 + 1024, 64:70].rearrange(
                            "(g p) c -> p g c", p=128),
                        u3[:, :, :])

            # ---------------- Phase 2: edge windows ----------------
            nwin = 0 if os.environ.get("SKIP_P2") else W
            with tc.tile_pool(name="p2", bufs=3) as p2, \
                 tc.tile_pool(name="p2g", bufs=2) as p2g, \
                 tc.tile_pool(name="p2w", bufs=2) as p2w, \
                 tc.tile_pool(name="pstr", bufs=2, space="PSUM") as pstr, \
                 tc.tile_pool(name="psus", bufs=2, space="PSUM") as psus, \
                 tc.tile_pool(name="psag", bufs=2, space="PSUM") as psag, \
                 tc.tile_pool(name="psrs", bufs=1, space="PSUM") as psrs, \
                 tc.tile_pool(name="pse", bufs=1, space="PSUM") as pse:
                for w in range(nwin):
                    emp = p2w.tile([128, K * 6], f32, tag="emp", name="emp")
                    nc.sync.dma_start(emp[:, :], d_emp.ap()[w, :, :])
                    p3 = emp[:, :].rearrange("p (k c) -> p k c", c=6)
                    ems = p2w.tile([128, K], bf16, tag="ems", name="ems")
                    nc.sync.dma_start(ems[:, :], d_ems.ap()[w, :, :])
                    gi = p2w.tile([128, K * 8], i16, tag="gi", name="gi")
                    nc.sync.dma_start(gi[:, :], d_gi.ap()[w, :, :])
                    gia = gi[:, 0:KA * 8]
                    gib = gi[:, KA * 8:K * 8]
                    vwin = p2w.tile([128, 6], f32, tag="vwin", name="vwin")
                    nc.sync.dma_start(vwin[:, :], d_z.ap()[w * 128:w * 128 + 128, 64:70])
                    vwinb = p2w.tile([128, 6], bf16, tag="vwinb", name="vwinb")
                    nc.vector.tensor_copy(vwinb[:, :], vwin[:, :])

                    gbuf = p2g.tile([128, K, 128], f32, tag="gbuf", name="gbuf")
                    CH = 8
                    for c0 in range(0, KA, CH):
                        n = min(CH, KA - c0)
                        nc.gpsimd.dma_gather(
                            gbuf[:, c0:c0 + n, :], d_z.ap()[:, :],
                            gia[:, c0 * 8:(c0 + n) * 8], n * 128, n * 128, 128,
                            queue_num=(c0 // CH) % SWQ)
                    for c0 in range(0, KB, CH):
                        n = min(CH, KB - c0)
                        nc.gpsimd.dma_gather(
                            gbuf[:, KA + c0:KA + c0 + n, :], d_z.ap()[HALF:, :],
                            gib[:, c0 * 8:(c0 + n) * 8], n * 128, n * 128, 128,
                            queue_num=(KA // CH + c0 // CH + 1) % SWQ)

                    # one-hot [slot, K, src] in bf16 via broadcast compare
                    oh = p2w.tile([128, K, 128], bf16, tag="oh", name="oh")
                    srCb = ems[:, :].rearrange("p (k o) -> p k o", o=1)
                    nc.vector.tensor_tensor(
                        oh[:, :, :], cs["iotak"][:, :].rearrange(
                            "p (k s) -> p k s", s=128),
                        srCb.broadcast_to([128, K, 128]), AOp.is_equal)

                    # src-side u expansion: transpose one-hots, us = s01T @ vwin
                    us_ps = psus.tile([128, K * 6], f32, tag="us", name="us")
                    nb4 = -(-K // 4)
                    for b4 in range(nb4):
                        t0 = b4 * 4
                        nt = min(4, K - t0)
                        sT_ps = pstr.tile([128, 512], bf16, tag="sT", name="sT")
                        for t in range(nt):
                            nc.tensor.transpose(sT_ps[:, t * 128:(t + 1) * 128],
                                                oh[:, t0 + t, :], cs["identb"][:, :])
                        sT_sb = p2.tile([128, 512], bf16, tag="sTs", name="sTs")
                        nc.scalar.copy(sT_sb[:, 0:nt * 128], sT_ps[:, 0:nt * 128])
                        for t in range(nt):
                            nc.tensor.matmul(
                                us_ps[:, (t0 + t) * 6:(t0 + t) * 6 + 6],
                                sT_sb[:, t * 128:(t + 1) * 128], vwinb[:, :],
                                start=True, stop=True)

                    # q = sum_c (us + u_dst) * p ; w = exp(-0.5 q)
                    usum = p2w.tile([128, K * 6], f32, tag="usum", name="usum")
                    us3 = usum[:, :].rearrange("p (k c) -> p k c", c=6)
                    nc.vector.tensor_add(
                        us3[:, :, :],
                        us_ps[:, :].rearrange("p (k c) -> p k c", c=6),
                        gbuf[:, :, 64:70])
                    pu = p2w.tile([128, K * 6], f32, tag="pu", name="pu")
                    pu3 = pu[:, :].rearrange("p (k c) -> p k c", c=6)
                    nc.vector.tensor_mul(pu3[:, :, :], us3[:, :, :], p3[:, :, :])
                    qcol = p2w.tile([128, K], f32, tag="qcol", name="qcol")
                    nc.vector.tensor_reduce(
                        qcol[:, :].rearrange("p (k o) -> p k o", o=1),
                        pu3[:, :, :], AxL.X, AOp.add)
                    wcolb = p2w.tile([128, K], bf16, tag="wcolb", name="wcolb")
                    nc.scalar.activation(wcolb[:, :], qcol[:, :], AF.Exp, scale=-0.5)

                    # sw = one-hot * w  (bf16)
                    sw = p2w.tile([128, K, 128], bf16, tag="sw", name="sw")
                    wcb = wcolb[:, :].rearrange("p (k o) -> p k o", o=1)
                    nc.vector.tensor_tensor(sw[:, :, :], oh[:, :, :],
                                            wcb.broadcast_to([128, K, 128]), AOp.mult)

                    # weighted segment-sum in [feat, src] layout + rowsum
                    aggT = psag.tile([128, 128], f32, tag="aggT", name="aggT")
                    rs_ps = psrs.tile([1, 128], f32, tag="rs", name="rs")
                    for t in range(K):
                        gf = gbuf[:, t, 0:64].bitcast(bf16)
                        nc.tensor.matmul(aggT[:, :], gf, sw[:, t, :],
                                         start=(t == 0), stop=(t == K - 1))
                        nc.tensor.matmul(rs_ps[:, :], cs["onesc"][:, :], sw[:, t, :],
                                         start=(t == 0), stop=(t == K - 1))

                    # epilogue: out = rcp * (aggT.T @ Wt) + bias
                    aggTs = p2.tile([128, 128], f32, tag="aggTs", name="aggTs")
                    nc.scalar.copy(aggTs[:, :], aggT[:, :])
                    rs_sb = p2.tile([1, 128], f32, tag="rs_sb", name="rs_sb")
                    nc.scalar.copy(rs_sb[:, :], rs_ps[:, :])
                    rsT_ps = pse.tile([128, 128], f32, tag="pse_t", name="rsT_ps")
                    nc.tensor.transpose(rsT_ps[:, 0:1], rs_sb[:, :],
                                        cs["ident1"][:, :])
                    rse = p2.tile([128, 1], f32, tag="rse", name="rse")
                    nc.vector.tensor_scalar_add(rse[:, :], rsT_ps[:, 0:1], EPS)
                    rcp = p2.tile([128, 1], f32, tag="rcp", name="rcp")
                    nc.vector.reciprocal(rcp[:, :], rse[:, :])
                    out_ps = pse.tile([128, 128], f32, tag="pse_t", name="out_ps")
                    nc.tensor.matmul(out_ps[:, :], aggTs[:, :], cs["wt"][:, :],
                                     start=True, stop=True)
                    out_s = p2.tile([128, 128], f32, tag="outs", name="outs")
                    nc.vector.tensor_mul(out_s[:, :], out_ps[:, :],
                                         rcp[:, :].broadcast_to([128, 128]))
                    nc.vector.tensor_add(out_s[:, :], out_s[:, :], cs["biasbc"][:, :])
                    nc.sync.dma_start(d_out.ap()[w * 128:(w + 1) * 128, :], out_s[:, :])

    nc.compile()
    return nc


_cache = {}


def kernel(features, vertices, edges, faces, W1, b1, W2, b2, weights, bias,
           _trace=False):
    in_maps, KA, KB = host_prep(features, vertices, edges)
    consts = const_inputs(W1, b1, W2, b2, weights, bias, KA + KB)
    outverts = []
    for m in in_maps:
        outverts.append(m.pop("_outvert"))
        m.update(consts)
    key = (KA, KB)
    if key not in _cache:
        _cache[key] = build_nc(KA, KB)
    nc = _cache[key]
    res = run_bass_kernel_spmd(nc, in_maps, core_ids=list(range(NCORES)),
                               trace=_trace)
    out = np.empty((N, F), np.float32)
    for c in range(NCORES):
        o = res.results[c]["out"]
        ov = outverts[c]
        valid = ov >= 0
        out[ov[valid]] = o[valid]
    kernel.last_result = res
    return out


# revision 26
# speedup vs baseline: 1.1056x; 1.0387x over previous
"""Trainium2 Bass kernel for nn_MetricConv (GNN message passing with learned metric).

v2 redesign (8-core SPMD, vertex-sharded, bin-packed windows):
  out_i = sum_{e:src=i} w_e * feat[dst_e] @ W / (rowsum_i + eps) + bias
  w_e = exp(-0.5 * (u[src]+u[dst]) . p(t_e)),  u[v] = 6-vec of G[v]=M^T M

Per-core vertex table (DRAM, 512B rows): [feat bf16 (256B) | u f32 (24B) | pad].
Host packs feat-bf16 into the table input and PERMUTES rows so this core's own
vertices sit in rows 0..6272 in bin order (49 bins x 128 slots, load-balanced
by (degA, degB) so K = KA+KB is minimal). Device phase 1 computes u for every
row (bf16 MLP via DMA-transposed feature tiles) and writes u back into the
input table. Phase 2 per window: dma_gather dst rows, build one-hot via
tensor_tensor broadcast compare (fast DVE path), per-tile PE transpose for the
src-side u expansion, weighted segment-sum matmuls in bf16.
"""
import os
import numpy as np
import ml_dtypes

import concourse.bacc as bacc
import concourse.bass as bass
import concourse.tile as tile
import concourse.mybir as mybir
from concourse.bass_utils import run_bass_kernel_spmd

N, E_TOT, C, F, H = 50000, 800000, 128, 128, 32
EPS = 1e-8
NCORES = 8
NV = N // NCORES          # 6250
W = 49                    # bins/windows per core
NS = W * 128              # 6272 slots
NROWS = 50176             # table rows (49 x 1024); >= 6272 + 7*6250
HALF = 32768              # int16 gather index split point
f32 = mybir.dt.float32
bf16 = mybir.dt.bfloat16
i16 = mybir.dt.int16
AOp = mybir.AluOpType
AF = mybir.ActivationFunctionType
AxL = mybir.AxisListType

SWQ = int(os.environ.get("SWQ", "1"))  # SWDGE queues for gather DGE


def host_prep(features, vertices, edges):
    """Per-core: bin-pack srcs, permute vertex table, build gather/edge meta."""
    feats = np.asarray(features, np.float32)
    featsb = feats.astype(ml_dtypes.bfloat16)
    verts = np.asarray(vertices, np.float32)
    src = np.asarray(edges[0], np.int64).astype(np.int32)
    dst = np.asarray(edges[1], np.int64).astype(np.int32)
    t_all = verts[dst] - verts[src]
    p_all = np.empty((len(src), 6), np.float32)
    p_all[:, 0:3] = t_all * t_all
    p_all[:, 3] = t_all[:, 0] * t_all[:, 1]
    p_all[:, 4] = t_all[:, 0] * t_all[:, 2]
    p_all[:, 5] = t_all[:, 1] * t_all[:, 2]

    percore = []
    KAmax = KBmax = 1
    for c in range(NCORES):
        lo, hi = c * NV, (c + 1) * NV
        sel = np.nonzero((src >= lo) & (src < hi))[0]
        srcL = src[sel] - lo

        # --- bin-pack the 6250 srcs into 49 bins of <=128, balancing A and B
        # edge counts separately (A/B of an edge is fixed before packing: own
        # dst -> rows 0..NS < HALF -> A; other dst -> row NS + rotpos, A iff
        # rotpos < HALF - NS).
        dstE = dst[sel]
        ownD = (dstE >= lo) & (dstE < hi)
        rotpos = (dstE - hi) % N
        isAe = ownD | (rotpos < HALF - NS)
        degA = np.bincount(srcL[isAe], minlength=NV).astype(np.float64)
        degB = np.bincount(srcL[~isAe], minlength=NV).astype(np.float64)
        deg = degA + degB
        order = np.argsort(-deg, kind="stable")
        binof = np.full(NV, -1, np.int32)
        slotof = np.full(NV, -1, np.int32)
        loadsA = np.zeros(W)
        loadsB = np.zeros(W)
        cnts = np.zeros(W, np.int64)
        mA = max(degA.sum() / W, 1.0)
        mB = max(degB.sum() / W, 1.0)
        for v in order:
            cand = np.nonzero(cnts < 128)[0]
            score = np.maximum((loadsA[cand] + degA[v]) / mA,
                               (loadsB[cand] + degB[v]) / mB)
            b = cand[np.argmin(score)]
            binof[v] = b
            slotof[v] = cnts[b]
            cnts[b] += 1
            loadsA[b] += degA[v]
            loadsB[b] += degB[v]

        # vertex -> table row (this core's table)
        row_of = np.empty(N, np.int32)
        own = np.arange(lo, hi)
        row_of[own] = binof[own - lo] * 128 + slotof[own - lo]
        others = np.concatenate([np.arange(hi, N), np.arange(0, lo)])
        row_of[others] = NS + np.arange(len(others), dtype=np.int32)

        dstR = row_of[dst[sel]]
        isA = dstR < HALF
        win = binof[srcL]
        sslot = slotof[srcL]
        orderE = np.lexsort((~isA, win))
        percore.append((win[orderE], sslot[orderE], dstR[orderE], isA[orderE],
                        p_all[sel][orderE], row_of, binof, slotof))
        for w in range(W):
            m = win[orderE] == w
            na = int(np.count_nonzero(m & isA[orderE]))
            nb = int(np.count_nonzero(m)) - na
            KAmax = max(KAmax, -(-na // 128))
            KBmax = max(KBmax, -(-nb // 128))
    KA, KB = KAmax, KBmax
    K = KA + KB

    in_maps = []
    for c in range(NCORES):
        win, sslot, dstR, isA, pC, row_of, binof, slotof = percore[c]
        lo = c * NV
        # permuted feature table (packed bf16 in cols 0:64)
        vid = np.empty(NROWS, np.int64)
        vid[:] = -1
        # rows -> vertex
        rows = row_of  # vertex -> row
        vr = np.argsort(rows)  # vertices sorted by row
        vid[rows[vr]] = vr
        zinit = np.zeros((NROWS, 128), np.float32)
        valid = vid >= 0
        zinit[valid, 0:64] = featsb[vid[valid]].view(np.float32)

        gidx = np.zeros((W, K, 128), np.int16)
        srcrel = np.full((W, K, 128), 200.0, np.float32)
        pbuf = np.zeros((W, 128, K, 6), np.float32)
        for w in range(W):
            m = win == w
            for grp in range(2):
                g = m & (isA if grp == 0 else ~isA)
                idxs = np.nonzero(g)[0]
                n = len(idxs)
                if n == 0:
                    continue
                t0 = 0 if grp == 0 else KA
                off = 0 if grp == 0 else HALF
                ntile = -(-n // 128)
                pad = ntile * 128 - n
                di = np.concatenate([dstR[idxs] - off, np.zeros(pad, np.int32)])
                sr = np.concatenate([sslot[idxs],
                                     np.full(pad, 200, np.int32)]).astype(np.float32)
                pv = np.concatenate([pC[idxs], np.zeros((pad, 6), np.float32)], 0)
                gidx[w, t0:t0 + ntile] = di.reshape(ntile, 128).astype(np.int16)
                srcrel[w, t0:t0 + ntile] = sr.reshape(ntile, 128)
                pbuf[w, :, t0:t0 + ntile, :] = pv.reshape(ntile, 128, 6).transpose(1, 0, 2)

        def wrap(g2, ntile):
            n = ntile * 128
            gw = g2.reshape(W, n // 16, 16).transpose(0, 2, 1)
            return np.ascontiguousarray(np.tile(gw, (1, 8, 1)))
        gm = np.concatenate([wrap(gidx[:, :KA].reshape(W, KA * 128), KA),
                             wrap(gidx[:, KA:].reshape(W, KB * 128), KB)], axis=2)
        # out row (w*128+slot) -> vertex id (or -1)
        outvert = np.full(NS, -1, np.int64)
        ownv = np.arange(lo, lo + NV)
        outvert[rows[ownv]] = ownv
        in_maps.append({
            "ztab": zinit,
            "emp": np.ascontiguousarray(
                pbuf.reshape(W, 128, K * 6)),                    # [W,128,K*6] f32
            "ems": np.ascontiguousarray(
                srcrel.transpose(0, 2, 1)).astype(ml_dtypes.bfloat16),  # [W,128,K]
            "gidx": np.ascontiguousarray(gm),                    # [W,128,K*8] i16
        })
        in_maps[-1]["_outvert"] = outvert
    return in_maps, KA, KB


def const_inputs(W1, b1, W2, b2, weights, bias, K):
    ii = np.tile(np.arange(128, dtype=np.float32), K)
    return {
        "w1b": np.asarray(W1, np.float32).astype(ml_dtypes.bfloat16),   # [128,32]
        "b1c": np.asarray(b1, np.float32).reshape(H, 1),                # [32,1]
        "w2b": np.asarray(W2, np.float32).astype(ml_dtypes.bfloat16),   # [32,9]
        "b2bc": np.tile(np.asarray(b2, np.float32), (128, 8)),          # [128,72]
        "wt": np.asarray(weights, np.float32),                          # [128,128]
        "biasbc": np.tile(np.asarray(bias, np.float32), (128, 1)),      # [128,128]
        "identb": np.eye(128, dtype=np.float32).astype(ml_dtypes.bfloat16),
        "ident1": np.eye(1, dtype=np.float32),
        "iotak": np.tile(ii, (128, 1)).astype(ml_dtypes.bfloat16),      # [128,K*128]
        "onesc": np.ones((128, 1), np.float32).astype(ml_dtypes.bfloat16),
    }


def const_shapes(K):
    return {"w1b": ([C, H], bf16), "b1c": ([H, 1], f32), "w2b": ([H, 9], bf16),
            "b2bc": ([128, 72], f32), "wt": ([C, F], f32),
            "biasbc": ([128, F], f32), "identb": ([128, 128], bf16),
            "ident1": ([1, 1], f32),
            "iotak": ([128, K * 128], bf16), "onesc": ([128, 1], bf16)}


def build_nc(KA, KB):
    K = KA + KB
    nc = bacc.Bacc("TRN2", target_bir_lowering=False, debug=False,
                   num_devices=NCORES, num_swdge_queues=SWQ)

    d_z = nc.dram_tensor("ztab", [NROWS, C], f32, kind="ExternalInput")
    d_emp = nc.dram_tensor("emp", [W, 128, K * 6], f32, kind="ExternalInput")
    d_ems = nc.dram_tensor("ems", [W, 128, K], bf16, kind="ExternalInput")
    d_gi = nc.dram_tensor("gidx", [W, 128, K * 8], i16, kind="ExternalInput")
    CS = const_shapes(K)
    dc = {k: nc.dram_tensor(k, sh, dt, kind="ExternalInput")
          for k, (sh, dt) in CS.items()}
    d_out = nc.dram_tensor("out", [NS, F], f32, kind="ExternalOutput")
    zb = d_z.ap().bitcast(bf16)  # [NROWS, 256] bf16 view

    with tile.TileContext(nc) as tc:
        with tc.tile_pool(name="const", bufs=1) as cp:
            cs = {}
            for k, (sh, dt) in CS.items():
                cs[k] = cp.tile(sh, dt, tag=k, name=k)
                nc.sync.dma_start(cs[k][:, :], dc[k].ap()[:, :])

            # ---------------- Phase 1: compute u for all rows ----------------
            NCH = NROWS // 1024
            with tc.tile_pool(name="p1", bufs=3) as p1, \
                 tc.tile_pool(name="p1u", bufs=1) as p1u, \
                 tc.tile_pool(name="p1b", bufs=2) as p1b, \
                 tc.tile_pool(name="ps1", bufs=2, space="PSUM") as ps1, \
                 tc.tile_pool(name="psm", bufs=2, space="PSUM") as psm:
                # u accumulates in SBUF; all z-table u-writes are deferred past
                # the loop so chunk k's write never serializes chunk k+1's
                # feature read of the same tensor
                u_all = p1u.tile([128, NCH * 48], f32, tag="uall", name="uall")
                for chunk in range(NCH):
                    v0 = chunk * 1024
                    ftTt = p1.tile([128, 1024], bf16, tag="ftT", name="ftT")
                    nc.sync.dma_start_transpose(ftTt[:, :], zb[v0:v0 + 1024, 0:128])
                    ftT = ftTt[:, :]
                    mb = psm.tile([128, 72], f32, tag="mb", name="mb")
                    for hf in range(2):
                        hT_ps = ps1.tile([32, 512], f32, tag="hT", name="hT")
                        nc.tensor.matmul(hT_ps[:, :], cs["w1b"][:, :],
                                         ftT[:, hf * 512:(hf + 1) * 512],
                                         start=True, stop=True)
                        hTs = p1.tile([32, 512], bf16, tag="hTs", name="hTs")
                        nc.scalar.activation(hTs[:, :], hT_ps[:, :], AF.Relu,
                                             bias=cs["b1c"][:, :])
                        for g in range(4):
                            gg = hf * 4 + g
                            nc.tensor.matmul(mb[:, gg * 9:gg * 9 + 9],
                                             hTs[:, g * 128:(g + 1) * 128],
                                             cs["w2b"][:, :], start=True, stop=True)
                    # u from M batch: G = M^T M -> 6-vector (crosses doubled)
                    m_s = p1b.tile([128, 72], f32, tag="m", name="m")
                    nc.vector.tensor_add(m_s[:, :], mb[:, :], cs["b2bc"][:, :])
                    sq = p1b.tile([128, 72], f32, tag="sq", name="sq")
                    nc.vector.tensor_mul(sq[:, :], m_s[:, :], m_s[:, :])
                    u_t = u_all[:, chunk * 48:(chunk + 1) * 48]
                    u3 = u_t[:, :].rearrange("p (g c) -> p g c", c=6)
                    s3 = sq[:, :].rearrange("p (g c) -> p g c", c=9)
                    nc.vector.tensor_add(u3[:, :, 0:3], s3[:, :, 0:3], s3[:, :, 3:6])
                    nc.vector.tensor_add(u3[:, :, 0:3], u3[:, :, 0:3], s3[:, :, 6:9])
                    m4 = m_s[:, :].rearrange("p (g k i) -> p g k i", k=3, i=3)
                    u4 = u_t[:, :].rearrange("p (g c i) -> p g c i", c=6, i=1)
                    ct = p1b.tile([128, 24], f32, tag="ct", name="ct")
                    ct4 = ct[:, :].rearrange("p (g k i) -> p g k i", k=3, i=1)
                    for ci, (i, j) in enumerate([(0, 1), (0, 2), (1, 2)]):
                        nc.vector.tensor_mul(ct4[:, :, :, :], m4[:, :, :, i:i + 1],
                                             m4[:, :, :, j:j + 1])
                        nc.vector.tensor_add(u4[:, :, 3 + ci:4 + ci, :],
                                             ct4[:, :, 0:1, :], ct4[:, :, 1:2, :])
                        nc.vector.tensor_add(u4[:, :, 3 + ci:4 + ci, :],
                                             u4[:, :, 3 + ci:4 + ci, :],
                                             ct4[:, :, 2:3, :])
                    nc.vector.tensor_scalar_mul(u3[:, :, 3:6], u3[:, :, 3:6], 2.0)
                for chunk in range(NCH):
                    v0 = chunk * 1024
                    nc.sync.dma_start(
                        d_z.ap()[v0:v0 + 1024, 64:70].rearrange(
                            "(g p) c -> p g c", p=128),
                        u_all[:, chunk * 48:(chunk + 1) * 48].rearrange(
                            "p (g c) -> p g c", c=6))

            # ---------------- Phase 2: edge windows ----------------
            nwin = 0 if os.environ.get("SKIP_P2") else W
            with tc.tile_pool(name="p2", bufs=3) as p2, \
                 tc.tile_pool(name="p2g", bufs=2) as p2g, \
                 tc.tile_pool(name="p2w", bufs=2) as p2w, \
                 tc.tile_pool(name="pstr", bufs=2, space="PSUM") as pstr, \
                 tc.tile_pool(name="psus", bufs=2, space="PSUM") as psus, \
                 tc.tile_pool(name="psag", bufs=2, space="PSUM") as psag, \
                 tc.tile_pool(name="psrs", bufs=1, space="PSUM") as psrs, \
                 tc.tile_pool(name="pse", bufs=1, space="PSUM") as pse:
                for w in range(nwin):
                    emp = p2w.tile([128, K * 6], f32, tag="emp", name="emp")
                    nc.sync.dma_start(emp[:, :], d_emp.ap()[w, :, :])
                    p3 = emp[:, :].rearrange("p (k c) -> p k c", c=6)
                    ems = p2w.tile([128, K], bf16, tag="ems", name="ems")
                    nc.sync.dma_start(ems[:, :], d_ems.ap()[w, :, :])
                    gi = p2w.tile([128, K * 8], i16, tag="gi", name="gi")
                    nc.sync.dma_start(gi[:, :], d_gi.ap()[w, :, :])
                    gia = gi[:, 0:KA * 8]
                    gib = gi[:, KA * 8:K * 8]
                    vwin = p2w.tile([128, 6], f32, tag="vwin", name="vwin")
                    nc.sync.dma_start(vwin[:, :], d_z.ap()[w * 128:w * 128 + 128, 64:70])
                    vwinb = p2w.tile([128, 6], bf16, tag="vwinb", name="vwinb")
                    nc.vector.tensor_copy(vwinb[:, :], vwin[:, :])

                    gbuf = p2g.tile([128, K, 128], f32, tag="gbuf", name="gbuf")
                    CH = 8
                    for c0 in range(0, KA, CH):
                        n = min(CH, KA - c0)
                        nc.gpsimd.dma_gather(
                            gbuf[:, c0:c0 + n, :], d_z.ap()[:, :],
                            gia[:, c0 * 8:(c0 + n) * 8], n * 128, n * 128, 128,
                            queue_num=(c0 // CH) % SWQ)
                    for c0 in range(0, KB, CH):
                        n = min(CH, KB - c0)
                        nc.gpsimd.dma_gather(
                            gbuf[:, KA + c0:KA + c0 + n, :], d_z.ap()[HALF:, :],
                            gib[:, c0 * 8:(c0 + n) * 8], n * 128, n * 128, 128,
                            queue_num=(KA // CH + c0 // CH + 1) % SWQ)

                    # one-hot [slot, K, src] in bf16 via broadcast compare
                    oh = p2w.tile([128, K, 128], bf16, tag="oh", name="oh")
                    srCb = ems[:, :].rearrange("p (k o) -> p k o", o=1)
                    nc.vector.tensor_tensor(
                        oh[:, :, :], cs["iotak"][:, :].rearrange(
                            "p (k s) -> p k s", s=128),
                        srCb.broadcast_to([128, K, 128]), AOp.is_equal)

                    # src-side u expansion: transpose one-hots, us = s01T @ vwin
                    us_ps = psus.tile([128, K * 6], f32, tag="us", name="us")
                    nb4 = -(-K // 4)
                    for b4 in range(nb4):
                        t0 = b4 * 4
                        nt = min(4, K - t0)
                        sT_ps = pstr.tile([128, 512], bf16, tag="sT", name="sT")
                        for t in range(nt):
                            nc.tensor.transpose(sT_ps[:, t * 128:(t + 1) * 128],
                                                oh[:, t0 + t, :], cs["identb"][:, :])
                        sT_sb = p2.tile([128, 512], bf16, tag="sTs", name="sTs")
                        nc.scalar.copy(sT_sb[:, 0:nt * 128], sT_ps[:, 0:nt * 128])
                        for t in range(nt):
                            nc.tensor.matmul(
                                us_ps[:, (t0 + t) * 6:(t0 + t) * 6 + 6],
                                sT_sb[:, t * 128:(t + 1) * 128], vwinb[:, :],
                                start=True, stop=True)

                    # q = sum_c (us + u_dst) * p ; w = exp(-0.5 q)
                    usum = p2w.tile([128, K * 6], f32, tag="usum", name="usum")
                    us3 = usum[:, :].rearrange("p (k c) -> p k c", c=6)
                    nc.vector.tensor_add(
                        us3[:, :, :],
                        us_ps[:, :].rearrange("p (k c) -> p k c", c=6),
                        gbuf[:, :, 64:70])
                    pu = p2w.tile([128, K * 6], f32, tag="pu", name="pu")
                    pu3 = pu[:, :].rearrange("p (k c) -> p k c", c=6)
                    nc.vector.tensor_mul(pu3[:, :, :], us3[:, :, :], p3[:, :, :])
                    qcol = p2w.tile([128, K], f32, tag="qcol", name="qcol")
                    nc.vector.tensor_reduce(
                        qcol[:, :].rearrange("p (k o) -> p k o", o=1),
                        pu3[:, :, :], AxL.X, AOp.add)
                    wcolb = p2w.tile([128, K], bf16, tag="wcolb", name="wcolb")
                    nc.scalar.activation(wcolb[:, :], qcol[:, :], AF.Exp, scale=-0.5)

                    # sw = one-hot * w  (bf16)
                    sw = p2w.tile([128, K, 128], bf16, tag="sw", name="sw")
                    wcb = wcolb[:, :].rearrange("p (k o) -> p k o", o=1)
                    nc.vector.tensor_tensor(sw[:, :, :], oh[:, :, :],
                                            wcb.broadcast_to([128, K, 128]), AOp.mult)

                    # weighted segment-sum in [feat, src] layout + rowsum
                    aggT = psag.tile([128, 128], f32, tag="aggT", name="aggT")
                    rs_ps = psrs.tile([1, 128], f32, tag="rs", name="rs")
                    for t in range(K):
                        gf = gbuf[:, t, 0:64].bitcast(bf16)
                        nc.tensor.matmul(aggT[:, :], gf, sw[:, t, :],
                                         start=(t == 0), stop=(t == K - 1))
                        nc.tensor.matmul(rs_ps[:, :], cs["onesc"][:, :], sw[:, t, :],
                                         start=(t == 0), stop=(t == K - 1))

                    # epilogue: out = rcp * (aggT.T @ Wt) + bias
                    aggTs = p2.tile([128, 128], f32, tag="aggTs", name="aggTs")
                    nc.scalar.copy(aggTs[:, :], aggT[:, :])
                    rs_sb = p2.tile([1, 128], f32, tag="rs_sb", name="rs_sb")
                    nc.scalar.copy(rs_sb[:, :], rs_ps[:, :])
                    rsT_ps = pse.tile([128, 128], f32, tag="pse_t", name="rsT_ps")
                    nc.tensor.transpose(rsT_ps[:, 0:1], rs_sb[:, :],
                                        cs["ident1"][:, :])
                    rse = p2.tile([128, 1], f32, tag="rse", name="rse")
                    nc.vector.tensor_scalar_add(rse[:, :], rsT_ps[:, 0:1], EPS)
                    rcp = p2.tile([128, 1], f32, tag="rcp", name="rcp")
                    nc.vector.reciprocal(rcp[:, :], rse[:, :])
                    out_ps = pse.tile([128, 128], f32, tag="pse_t", name="out_ps")
                    nc.tensor.matmul(out_ps[:, :], aggTs[:, :], cs["wt"][:, :],
                                     start=True, stop=True)
                    out_s = p2.tile([128, 128], f32, tag="outs", name="outs")
                    nc.vector.tensor_mul(out_s[:, :], out_ps[:, :],
                                         rcp[:, :].broadcast_to([128, 128]))
                    nc.vector.tensor_add(out_s[:, :], out_s[:, :], cs["biasbc"][:, :])
                    nc.sync.dma_start(d_out.ap()[w * 128:(w + 1) * 128, :], out_s[:, :])

    nc.compile()
    return nc


_cache = {}


def kernel(features, vertices, edges, faces, W1, b1, W2, b2, weights, bias,
           _trace=False):
    in_maps, KA, KB = host_prep(features, vertices, edges)
    consts = const_inputs(W1, b1, W2, b2, weights, bias, KA + KB)
    outverts = []
    for m in in_maps:
        outverts.append(m.pop("_outvert"))
        m.update(consts)
    key = (KA, KB)
    if key not in _cache:
        _cache[key] = build_nc(KA, KB)
    nc = _cache[key]
    res = run_bass_kernel_spmd(nc, in_maps, core_ids=list(range(NCORES)),
                               trace=_trace)
    out = np.empty((N, F), np.float32)
    for c in range(NCORES):
        o = res.results[c]["out"]
        ov = outverts[c]
        valid = ov >= 0
        out[ov[valid]] = o[valid]
    kernel.last_result = res
    return out
